# revision 14
# baseline (speedup 1.0000x reference)
"""Multi-head attention block (QKV proj -> softmax attention -> out proj) for
Trainium2, SPMD across 8 NeuronCores.

Sharding: batch (B=2) x head-groups (4 groups of 4 heads). Core c handles
batch c//4 and heads [4*(c%4), 4*(c%4)+4). Each core computes its partial
output contribution (context @ wo_slice.T); the host sums the 4 head-group
partials per batch (tensor-parallel row-sharded wo => the all-reduce is the
host-side gather).

All matmuls run in bf16 with fp32 PSUM accumulation. Softmax runs in fp32
out of PSUM (exp on the scalar engine, row-sum + reciprocal on DVE); the
probability transposes needed for the PV matmul go through the DMA X-bar
(SBUF->SBUF block transpose) instead of the PE, keeping the tensor engine
free for real matmuls.

Per-core kernel layout (everything [partition=128, free]):
  xT   [2048, 2048] bf16   x[b].T             (feature k on partitions)
  wqT/wkT/wvT [2048, 512]  w[heads_slice].T   (k on partitions)
  woT  [512, 2048]  bf16   wo[:, slice].T     (local d on partitions)
  out  [2048, 2048] fp32   partial output for batch b
"""

import sys

if "/opt/trn_rl_repo" not in sys.path:
    sys.path.insert(0, "/opt/trn_rl_repo")

from contextlib import ExitStack

import ml_dtypes
import numpy as np

import concourse.bacc as bacc
import concourse.tile as tile
from concourse import mybir
from concourse.bass_utils import run_bass_kernel_spmd

BF16 = mybir.dt.bfloat16
F32 = mybir.dt.float32

B, S, DIM = 2, 2048, 2048
HEADS, HD = 16, 128
P = 128
N_CORES = 8
HGROUPS = 4  # head groups (second shard axis is batch)
HPC = HEADS // HGROUPS  # heads per core = 4
DL = HPC * HD  # local head dims per core = 512
SCALE = 1.0 / float(np.sqrt(HD))

NK = DIM // P  # 16 contraction tiles for the projections
NM = S // 512  # 4 m-chunks (tokens)
NQ = S // P  # 16 q tiles
NN = S // P  # 16 kv tiles
NE = DIM // 512  # 4 output-dim chunks

_PROGRAM_CACHE = {}


def _emit(nc, tc, xT, wqT, wkT, wvT, woT, maskf, out):
    with_mask = maskf is not None
    with ExitStack() as octx:
        planes = octx.enter_context(tc.tile_pool(name="planes", bufs=1))
        q_sb = [planes.tile([P, S], BF16, tag=f"q{h}", name=f"q{h}") for h in range(HPC)]
        k_sb = [planes.tile([P, S], BF16, tag=f"k{h}", name=f"k{h}") for h in range(HPC)]
        ctx_sb = [planes.tile([P, S], BF16, tag=f"ctx{h}", name=f"ctx{h}") for h in range(HPC)]

        vv_pool = octx.enter_context(tc.tile_pool(name="vv", bufs=1))

        # ---------------- Phase 1: QKV projections ----------------
        with ExitStack() as ctx:
            wpool = ctx.enter_context(tc.tile_pool(name="wqkv", bufs=1))
            vT_sb = [wpool.tile([P, S], BF16, tag=f"vt{h}", name=f"vt{h}")
                     for h in range(HPC)]
            w_sb = {}
            for name in ("q", "k", "v"):
                w_sb[name] = wpool.tile([P, NK * DL], BF16, tag=f"w{name}",
                                        name=f"w{name}")
            xpool = ctx.enter_context(tc.tile_pool(name="xt", bufs=2 * NK))
            pq = ctx.enter_context(tc.tile_pool(name="ps_qkv", bufs=4, space="PSUM"))

            for mc in range(NM):
                xts = []
                for kt in range(NK):
                    t = xpool.tile([P, 512], BF16, tag="xt")
                    nc.sync.dma_start(
                        t[:], xT[kt * P : (kt + 1) * P, mc * 512 : (mc + 1) * 512]
                    )
                    xts.append(t)
                    if mc == 0:
                        # q weights interleave with the first x tiles so the
                        # first accumulation groups start early; k/v follow
                        nc.sync.dma_start(
                            w_sb["q"][:, kt * DL : (kt + 1) * DL],
                            wqT[kt * P : (kt + 1) * P, :],
                        )
                if mc == 0:
                    for kt in range(NK):
                        for name, srct in (("k", wkT), ("v", wvT)):
                            nc.sync.dma_start(
                                w_sb[name][:, kt * DL : (kt + 1) * DL],
                                srct[kt * P : (kt + 1) * P, :],
                            )
                for name, plane_list in (("q", q_sb), ("k", k_sb), ("v", vT_sb)):
                    for h in range(HPC):
                        ps = pq.tile([P, 512], F32, tag="ps")
                        for kt in range(NK):
                            nc.tensor.matmul(
                                ps[:],
                                w_sb[name][:, kt * DL + h * P : kt * DL + (h + 1) * P],
                                xts[kt][:],
                                start=(kt == 0),
                                stop=(kt == NK - 1),
                            )
                        nc.any.tensor_copy(
                            plane_list[h][:, mc * 512 : (mc + 1) * 512], ps[:]
                        )

            # v tiles to [kv, d] orientation via DMA x-bar transpose, all heads
            vvs = []
            for h in range(HPC):
                vv = vv_pool.tile([P, NN, P], BF16, tag=f"vv{h}", name=f"vv{h}")
                nc.sync.dma_start(vv[:], vT_sb[h][:], transpose=True)
                vvs.append(vv)

        # ------- Phase 2+3: attention (block-pipelined) + out projection -------
        with ExitStack() as ctx:
            wopool = ctx.enter_context(tc.tile_pool(name="wo", bufs=1))
            wo_sb = [wopool.tile([P, DIM], BF16, tag=f"wo{h}", name=f"wo{h}")
                     for h in range(HPC)]
            for h in range(HPC):
                nc.gpsimd.dma_start(wo_sb[h][:], woT[h * P : (h + 1) * P, :])

            praw_pool = ctx.enter_context(tc.tile_pool(name="praw", bufs=6))
            pT_pool = ctx.enter_context(tc.tile_pool(name="pT", bufs=2))
            stats = ctx.enter_context(tc.tile_pool(name="stats", bufs=12))
            opool = ctx.enter_context(tc.tile_pool(name="ob", bufs=3))
            if with_mask:
                mpool = ctx.enter_context(tc.tile_pool(name="mask", bufs=4))
            ps_s = ctx.enter_context(tc.tile_pool(name="ps_s", bufs=2, space="PSUM"))
            ps_cd = ctx.enter_context(tc.tile_pool(name="ps_cd", bufs=4, space="PSUM"))

            def s_qtile(jb, h, t, praw):
                """scores + softmax for one q-tile of 128 rows."""
                qoff = jb * 512 + t * P
                dsum = stats.tile([P, 1], F32, tag="dsum")
                for half in range(2):  # kv chunks of 1024
                    ps = ps_s.tile([P, 1024], F32, tag="ps_s")
                    for sub in range(2):
                        nj = half * 2 + sub
                        nc.tensor.matmul(
                            ps[:, sub * 512 : (sub + 1) * 512],
                            q_sb[h][:, qoff : qoff + P],
                            k_sb[h][:, nj * 512 : (nj + 1) * 512],
                            start=True,
                            stop=True,
                        )
                    if with_mask:
                        mt = mpool.tile([P, 1024], F32, tag="mt")
                        nc.gpsimd.dma_start(
                            mt[:],
                            maskf[qoff : qoff + P, half * 1024 : (half + 1) * 1024],
                        )
                        nc.vector.tensor_add(ps[:], ps[:], mt[:])
                    nc.scalar.activation(
                        praw[:, half * 1024 : (half + 1) * 1024],
                        ps[:],
                        mybir.ActivationFunctionType.Exp,
                        scale=SCALE,
                        accum_out=(dsum[:] if half == 0 else None),
                    )
                red = stats.tile([P, 1], F32, tag="red")
                nc.vector.tensor_reduce(
                    red[:], praw[:, 1024:2048], axis=mybir.AxisListType.X,
                    op=mybir.AluOpType.add,
                )
                den = stats.tile([P, 1], F32, tag="den")
                nc.vector.tensor_add(den[:], dsum[:], red[:])
                rec = stats.tile([P, 1], F32, tag="rec")
                nc.vector.reciprocal(rec[:], den[:])
                nc.vector.tensor_scalar_mul(praw[:], praw[:], rec[:])

            def pv(jb, h, pT):
                psc = ps_cd.tile([P, 512], F32, tag="ps_cd")
                for nt in range(NN):
                    nc.tensor.matmul(
                        psc[:],
                        vvs[h][:, nt, :],
                        pT[:, nt, :, :],
                        start=(nt == 0),
                        stop=(nt == NN - 1),
                    )
                nc.any.tensor_copy(ctx_sb[h][:, jb * 512 : (jb + 1) * 512], psc[:])

            def d_group(tt, ec):
                ps = ps_cd.tile([P, 512], F32, tag="ps_cd")
                for h in range(HPC):
                    nc.tensor.matmul(
                        ps[:],
                        ctx_sb[h][:, tt * P : (tt + 1) * P],
                        wo_sb[h][:, ec * 512 : (ec + 1) * 512],
                        start=(h == 0),
                        stop=(h == HPC - 1),
                    )
                ob = opool.tile([P, 512], F32, tag="ob")
                nc.any.tensor_copy(ob[:], ps[:])
                nc.gpsimd.dma_start(
                    out[tt * P : (tt + 1) * P, ec * 512 : (ec + 1) * 512], ob[:]
                )

            # Per q-block: scores/softmax stretches per head (ACT-paced) are
            # padded with out-projection groups of the previous block so the
            # PE never idles long enough for HAM to re-throttle; each head's
            # PV lands at the end of the next head's stretch (x-bar transpose
            # latency hidden); the remaining out-proj groups + last PV fill
            # the block tail.
            for jb in range(4):
                d_list = (
                    [(tt, ec) for tt in range(4 * (jb - 1), 4 * jb - 4 + 4)
                     for ec in range(NE)]
                    if jb > 0 else []
                )
                di = 0
                prev = None
                for h in range(HPC):
                    pT = pT_pool.tile([P, NN, 4, P], BF16, tag="pT", name="pT")
                    for t in range(4):
                        if di < len(d_list):
                            d_group(*d_list[di])
                            di += 1
                        praw = praw_pool.tile([P, S], BF16, tag="praw", name="praw")
                        s_qtile(jb, h, t, praw)
                        nc.sync.dma_start(pT[:, :, t, :], praw[:], transpose=True)
                    if prev is not None:
                        pv(jb, prev[0], prev[1])
                    prev = (h, pT)
                pv(jb, prev[0], prev[1])
                while di < len(d_list):
                    d_group(*d_list[di])
                    di += 1
            for tt in range(12, 16):
                for ec in range(NE):
                    d_group(tt, ec)


def _build(with_mask: bool):
    nc = bacc.Bacc("TRN2")
    xT = nc.dram_tensor("xT", [DIM, S], BF16, kind="ExternalInput")
    wqT = nc.dram_tensor("wqT", [DIM, DL], BF16, kind="ExternalInput")
    wkT = nc.dram_tensor("wkT", [DIM, DL], BF16, kind="ExternalInput")
    wvT = nc.dram_tensor("wvT", [DIM, DL], BF16, kind="ExternalInput")
    woT = nc.dram_tensor("woT", [DL, DIM], BF16, kind="ExternalInput")
    maskf = (
        nc.dram_tensor("maskf", [S, S], F32, kind="ExternalInput")
        if with_mask
        else None
    )
    out = nc.dram_tensor("out", [S, DIM], F32, kind="ExternalOutput")
    with tile.TileContext(nc) as tc:
        _emit(nc, tc, xT, wqT, wkT, wvT, woT, maskf, out)
    nc.finalize()
    return nc


def _get_program(with_mask: bool):
    if with_mask not in _PROGRAM_CACHE:
        _PROGRAM_CACHE[with_mask] = _build(with_mask)
    return _PROGRAM_CACHE[with_mask]


def _prep_in_maps(x, mask, wq, wk, wv, wo, with_mask):
    bf = ml_dtypes.bfloat16
    f32 = np.float32
    xTs = [np.ascontiguousarray(x[b].T.astype(bf)) for b in range(B)]
    if with_mask:
        maskf = np.ascontiguousarray((mask[0, 0].astype(f32) / SCALE))
    in_maps = []
    for c in range(N_CORES):
        b = c // HGROUPS
        g = c % HGROUPS
        sl = slice(g * DL, (g + 1) * DL)
        m = {
            "xT": xTs[b],
            "wqT": np.ascontiguousarray(wq[sl, :].T.astype(bf)),
            "wkT": np.ascontiguousarray(wk[sl, :].T.astype(bf)),
            "wvT": np.ascontiguousarray(wv[sl, :].T.astype(bf)),
            "woT": np.ascontiguousarray(wo[:, sl].T.astype(bf)),
        }
        if with_mask:
            m["maskf"] = maskf
        in_maps.append(m)
    return in_maps


def run_sharded(x, mask, wq, wk, wv, wo, trace=False, trace_kwargs=None):
    """Run the SPMD kernel; returns (full_output, BassKernelResults)."""
    with_mask = bool(np.any(np.asarray(mask)))
    nc = _get_program(with_mask)
    in_maps = _prep_in_maps(
        np.asarray(x), np.asarray(mask), np.asarray(wq), np.asarray(wk),
        np.asarray(wv), np.asarray(wo), with_mask,
    )
    kw = {}
    if trace:
        kw["trace"] = True
        if trace_kwargs:
            kw["trace_kwargs"] = trace_kwargs
    res = run_bass_kernel_spmd(nc, in_maps, list(range(N_CORES)), **kw)
    out = np.zeros((B, S, DIM), np.float32)
    for c in range(N_CORES):
        out[c // HGROUPS] += res.results[c]["out"]
    return out, res


def kernel(**inputs):
    out, _ = run_sharded(
        inputs["x"], inputs["mask"], inputs["wq"], inputs["wk"], inputs["wv"],
        inputs["wo"],
    )
    return out
